# revision 10
# baseline (speedup 1.0000x reference)
"""Birth-death interval loss on 8 trn2 NeuronCores.

Data-parallel over batch: core i handles batches [2i, 2i+2) = 16 (b,c) images.
Host repacks both interval tensors per image into an SoA int32 layout
(per partition: y[512] | x[512], births then deaths) so each image needs one
4KB-per-partition DMA and one contiguous STT (flat = y*512 + x). One indirect
DMA (SWDGE gather, one 4B descriptor per endpoint) fetches all 65536 endpoint
values per image straight from the image in DRAM (element_offset selects the
image). DVE computes (birth-death)^2 partial sums. Host sums per-core partials.
"""
import os, sys, types

sys.path.insert(0, "/opt/trn_rl_repo")
sys.path.insert(0, "/root/.axon_site/trn_agent_boot")

import numpy as np


def _setup_env():
    import antenv  # noqa: F401

    if "antenv.axon_hooks" not in sys.modules:
        mod = types.ModuleType("antenv.axon_hooks")
        mod._hook = None
        mod.set_axon_ntff_profile_hook = lambda h: setattr(mod, "_hook", h)
        mod.get_axon_ntff_profile_hook = lambda: mod._hook
        sys.modules["antenv.axon_hooks"] = mod
        try:
            from trn_boot import _ntff_profile_via_ctypes

            mod._hook = _ntff_profile_via_ctypes("/opt/axon/libaxon_pjrt.so")
        except Exception:
            pass

    import concourse.tile as tile
    from concourse.vector_clock import ScopedClock
    from bass_rust import VectorClock

    def _split_drain_and_barrier(self, tick_clock, wait_clock):
        vals = list(tick_clock.global_clock)
        nz = [(i, v) for i, v in enumerate(vals) if v > 0]
        chunks = [nz[i : i + 1] for i in range(len(nz))] or [[]]
        for chunk in chunks:
            sub = [0] * len(vals)
            for i, v in chunk:
                sub[i] = v
            drain_inst = self.nc.sync.drain()
            wait_clock.add_sem_waits(
                drain_inst.ins, ScopedClock({None: VectorClock(sub)})
            )
        self.nc.all_engine_barrier()
        assert self.sems is not None
        popped = self.nc._tile_sem_poison_stack.pop()
        assert popped is self._sem_poison
        self.nc.clear_and_free_semaphores(list(self.sems.allocated().values()))
        self.nc.all_engine_barrier()

    tile.TileContext._drain_and_barrier = _split_drain_and_barrier


def _split_waits(nc):
    from concourse import mybir

    ctr = [0]
    for f in nc.m.functions:
        for bb in f.blocks:
            new = []
            changed = False
            for inst in bb.instructions:
                si = inst.sync_info
                if si is not None and len(si.on_wait) > 1:
                    waits = list(si.on_wait)
                    for w in waits[:-1]:
                        ctr[0] += 1
                        new.append(
                            mybir.InstEventSemaphore(
                                name=f"I-wsplit{ctr[0]}",
                                ins=[], outs=[], engine=inst.engine,
                                sync_info=mybir.SyncInfo(on_wait=[w], on_update=[]),
                            )
                        )
                    inst.sync_info = mybir.SyncInfo(
                        on_wait=waits[-1:], on_update=list(si.on_update)
                    )
                    changed = True
                new.append(inst)
            if changed:
                bb.instructions = new


NIMG = 16  # 2 batches x 8 channels per core
IMG_ELEMS = 512 * 512
PRED_ELEMS = NIMG * IMG_ELEMS

_BUILT = None


def _build():
    global _BUILT
    if _BUILT is not None:
        return _BUILT
    _setup_env()
    import concourse.bass as bass
    import concourse.tile as tile
    from concourse import mybir
    from contextlib import ExitStack

    f32 = mybir.dt.float32
    i32 = mybir.dt.int32
    Alu = mybir.AluOpType

    nc = bass.Bass("TRN2", target_bir_lowering=False, debug=False, num_devices=8)
    pred_d = nc.dram_tensor("pred", [PRED_ELEMS, 1], f32, kind="ExternalInput").ap()
    ivp_d = nc.dram_tensor("ivp", [NIMG, 128, 1024], i32, kind="ExternalInput").ap()
    out_d = nc.dram_tensor("out", [128, NIMG], f32, kind="ExternalOutput").ap()
    dump = bool(int(os.environ.get("BDL_DUMP", "0")))
    if dump:
        fdump_d = nc.dram_tensor(
            "fdump", [NIMG, 128, 512], i32, kind="ExternalOutput").ap()
        vdump_d = nc.dram_tensor(
            "vdump", [NIMG, 128, 512], f32, kind="ExternalOutput").ap()

    with tile.TileContext(nc) as tc, ExitStack() as ctx:
        cpool = ctx.enter_context(tc.tile_pool(name="c", bufs=1))
        recp = ctx.enter_context(tc.tile_pool(name="rec", bufs=6))
        idxp = ctx.enter_context(tc.tile_pool(name="idx", bufs=4))
        vp = ctx.enter_context(tc.tile_pool(name="v", bufs=4))
        dp = ctx.enter_context(tc.tile_pool(name="d", bufs=3))

        acc = cpool.tile([128, NIMG], f32, tag="acc")

        for t in range(NIMG):
            # per partition: y[512] | x[512] (births 0:256, deaths 256:512 in
            # each half, both interval tensors' records concatenated)
            recs = recp.tile([128, 1024], i32, tag="recs")
            nc.sync.dma_start(recs[:], ivp_d[t])

            flat = idxp.tile([128, 512], i32, tag="flat")
            nc.vector.scalar_tensor_tensor(
                flat[:], recs[:, :512], 512, recs[:, 512:],
                op0=Alu.mult, op1=Alu.add)

            # gather: vals[p, j] = pred[t*IMG + flat[p, j]]
            vals = vp.tile([128, 512], f32, tag="vals")
            nc.gpsimd.indirect_dma_start(
                out=vals[:],
                out_offset=None,
                in_=pred_d[:],
                in_offset=bass.IndirectOffsetOnAxis(ap=flat[:], axis=0),
                element_offset=t * IMG_ELEMS,
            )

            if dump:
                nc.sync.dma_start(fdump_d[t], flat[:])
                nc.sync.dma_start(vdump_d[t], vals[:])

            dt_ = dp.tile([128, 256], f32, tag="dt")
            nc.vector.tensor_sub(dt_[:], vals[:, :256], vals[:, 256:])
            dsq = dp.tile([128, 256], f32, tag="dsq")
            nc.vector.scalar_tensor_tensor(
                dsq[:], dt_[:], 1.0, dt_[:],
                op0=Alu.mult, op1=Alu.mult,
                accum_out=acc[:, t : t + 1])

        nc.sync.dma_start(out_d[:], acc[:])

    from concourse.library_overlay import lower_extended_insts

    lower_extended_insts(nc)
    _split_waits(nc)
    _BUILT = nc
    return nc


def _pack_intervals(iv0v, iv1v):
    """iv*v: (16, 8, 16384, 8) int32 views of the int64 tensors (lo words at
    even slots). Returns (16, NIMG, 128, 1024) int32: per core, image, partition:
    y[512] | x[512] with each half laid out [births 256 | deaths 256]."""
    # concat both tensors' records: (16, 8, 32768, 8)
    iv = np.concatenate([iv0v, iv1v], axis=2)
    y0 = iv[..., 0].reshape(16, 8, 128, 256)
    x0 = iv[..., 2].reshape(16, 8, 128, 256)
    y1 = iv[..., 4].reshape(16, 8, 128, 256)
    x1 = iv[..., 6].reshape(16, 8, 128, 256)
    # (16, 8, 128, 4, 256): y0 | y1 | x0 | x1
    packed = np.stack([y0, y1, x0, x1], axis=3)
    packed = packed.reshape(16, 8, 128, 1024)
    # core i: batches 2i, 2i+1 -> (8 cores, 16 images, 128, 1024)
    packed = packed.reshape(8, 2 * 8, 128, 1024)
    return np.ascontiguousarray(packed)


def kernel(prediction, intervals_comp_0, intervals_comp_1):
    nc = _build()
    from concourse.bass_utils import run_bass_kernel_spmd

    pred = np.ascontiguousarray(np.asarray(prediction, dtype=np.float32))
    iv0 = np.ascontiguousarray(np.asarray(intervals_comp_0)).astype(np.int64, copy=False)
    iv1 = np.ascontiguousarray(np.asarray(intervals_comp_1)).astype(np.int64, copy=False)
    iv0v = iv0.view(np.int32).reshape(16, 8, 16384, 8)
    iv1v = iv1.view(np.int32).reshape(16, 8, 16384, 8)
    ivp = _pack_intervals(iv0v, iv1v)

    in_maps = []
    for i in range(8):
        in_maps.append({
            "pred": pred[2 * i : 2 * i + 2].reshape(PRED_ELEMS, 1),
            "ivp": ivp[i],
        })

    trace = bool(int(os.environ.get("BDL_TRACE", "0")))
    res = run_bass_kernel_spmd(nc, in_maps, list(range(8)), trace=trace)
    kernel.last_result = res
    if trace:
        print(f"HW exec time: {res.exec_time_ns} ns", flush=True)

    total = np.float64(0.0)
    for i in range(8):
        total += np.asarray(res.results[i]["out"], dtype=np.float64).sum()
    return np.float32(total / 16.0)
